# revision 32
# baseline (speedup 1.0000x reference)
"""Trainium2 Bass kernel for MockMobGatedDeltaNetMoE (v17, ~453us from 802us).

Sharding: head-parallel over H=8 heads, one head per NeuronCore.
Each core computes its head's full contribution; the host sums the 8
partial output projections (per-core token permutations undone on host).

Key design points (history: v8 802us -> v9 552 -> v11 495 -> v15 457 -> v17 453):
 - Routing (softmax top-2 over the 4 routed experts) runs on the HOST in
   f64 from logits = hs @ (Wq_head @ Wgate); the device receives combine
   weights rw, spur-correction selectors nsel, and broadcast key masks.
   This removes 768 LDWEIGHTS-bound tiny matmuls (~75us) and halves the
   hidden-state DMA (single f16 tensor).
 - Fixed expert windows: each routed expert owns a constant 6-of-8
   key-chunk window (identical across cores, so one SPMD program).  The
   host solves a tiny exact max-flow per (core, batch) assigning every
   token to a 128-chunk inside BOTH of its selected experts' windows,
   then permutes tokens accordingly.  Scores/exp/combine for a routed
   expert only touch its window (packed PSUM layout); the 256 keys
   outside the window are all masked and contribute exactly exp(0)=1
   each to the softmax denominator (+256 constant).  Tiles outside an
   expert's window skip that expert entirely (5 of 6 experts per tile).
 - Score fusion: S_r = q @ (Wq_exp_r @ Wk_exp_r^T) @ k^T with M_r fused
   on host; masked keys give exp(0)=1 (reference semantics) and their
   masked-v contribution is removed by a rank-4 spur correction.
 - One merged exp per (query-tile, expert) with accum_out denominator;
   expert combine via diagonal matmuls (transpose+scale+accumulate in
   PSUM); one attention @ V matmul per query tile.
 - Emission is batch-interleaved (tb0,tb1 -> attn b0 -> tb2,tb3 ->
   attn b1 -> phase 4) over one shared 8-bank PSUM scheme; DMAs ship as
   multi-chunk [128, 2048] tiles via 3D access patterns (~0.6us issue
   cost each); Wo is DMA'd into the dead wg weight tiles after the last
   g-projection; one [128, 2048] store + one DMA per output tile.
"""

import numpy as np

import concourse.bass as bass
import concourse.bacc as bacc
import concourse.tile as tile
from concourse import mybir
from concourse.bass_utils import run_bass_kernel_spmd

F32 = mybir.dt.float32
F16 = mybir.dt.float16
ALU = mybir.AluOpType
ACTF = mybir.ActivationFunctionType

H, D, R, NE = 8, 256, 6, 4
HID, DV, T = 2048, 512, 2048
NB = 2
TB = T // NB
SCALE = 1.0 / 16.0

# ---- fixed expert-window scheme (uniform across cores -> one SPMD program).
# Each routed expert r owns a fixed 6-of-8 chunk window; the host permutes
# tokens (per core/batch) so every token lands in a chunk inside both of its
# selected experts' windows.  Keys outside W_r are all masked for r and
# contribute exactly exp(0)=1 each to the softmax denominator (+256 const).
WIN = {2: (0, 1, 2, 3, 4, 5), 3: (2, 3, 4, 5, 6, 7),
       4: (0, 1, 2, 3, 6, 7), 5: (0, 1, 4, 5, 6, 7)}
# score-matmul runs per routed expert: (packed_chunk_start, global_chunk_start, nchunks)
RUNS = {2: ((0, 0, 4), (4, 4, 2)), 3: ((0, 2, 4), (4, 6, 2)),
        4: ((0, 0, 4), (4, 6, 2)), 5: ((0, 0, 2), (2, 4, 2), (4, 6, 2))}
RUNS_FULL = ((0, 0, 4), (4, 4, 4))
# token runs (start, len) per routed expert's window, split at 512 boundaries
QRUNS = {2: ((0, 512), (512, 256)), 3: ((256, 256), (512, 512)),
         4: ((0, 512), (768, 256)), 5: ((0, 256), (512, 512))}
QRUNS_FULL = ((0, 512), (512, 512))
TILE_EXPERTS = [sorted(r for r in WIN if j in WIN[r]) for j in range(8)]
PAIRS = [(2, 3), (2, 4), (2, 5), (3, 4), (3, 5), (4, 5)]
# super-chunks (pairs of 128-chunks) allowed per expert pair
REGION = [(1, 2), (0, 1), (0, 2), (1, 3), (2, 3), (0, 3)]
NCOUT = 256.0   # keys outside a routed expert's window (all masked): 2 chunks


class Ctx:
    pass


def _emit_phase1_tb(nc, cx, tb, hst_t):
    """q/k/v/g projection chains for one 512-token block.

    hst_t: 8 tiles [128, 1024], chunk hc at hst_t[hc//2][:, (hc%2)*512:...]."""
    t0 = tb * 512

    def hst_mv(hc):
        return hst_t[hc // 2][:, (hc % 2) * 512:(hc % 2) * 512 + 512]

    # q/k -> transposed [d-chunk, token]; f0+f1 share one 2-bank psum
    for wt, dstT in ((cx.wq_t, cx.qT), (cx.wk_t, cx.kT)):
        ps = cx.ps.tile([128, 1024], F32, name="big", tag="big", bufs=2)
        for hc in range(16):
            wsl = wt[hc // 8]
            c0 = (hc % 8) * 256
            nc.tensor.matmul(ps[:, 0:512], wsl[:, c0:c0 + 128], hst_mv(hc),
                             start=(hc == 0), stop=(hc == 15))
            nc.tensor.matmul(ps[:, 512:1024], wsl[:, c0 + 128:c0 + 256], hst_mv(hc),
                             start=(hc == 0), stop=(hc == 15))
        nc.scalar.copy(dstT[:, t0:t0 + 512], ps[:, 0:512])
        nc.scalar.copy(dstT[:, T + t0:T + t0 + 512], ps[:, 512:1024])
    # v then g (separate passes; wg arrives after wv in the DMA stream)
    for wt, dst_sb, use_scalar in ((cx.wv_t, cx.v_sb, True), (cx.wg_t, cx.g_sb, False)):
        for half in range(2):
            ps = cx.ps.tile([128, 1024], F32, name="big", tag="big", bufs=2)
            for hc in range(16):
                wmv = wt[hc // 4][:, (hc % 4) * 512:(hc % 4) * 512 + 512]
                h0 = (hc % 2) * 512 + half * 256
                nc.tensor.matmul(ps[:, 0:512], hst_t[hc // 2][:, h0:h0 + 128],
                                 wmv, start=(hc == 0), stop=(hc == 15))
                nc.tensor.matmul(ps[:, 512:1024], hst_t[hc // 2][:, h0 + 128:h0 + 256],
                                 wmv, start=(hc == 0), stop=(hc == 15))
            tt = tb * 4 + half * 2
            for s in range(2):
                dst = dst_sb[:, (tt + s) * DV:(tt + s + 1) * DV]
                src = ps[:, s * 512:(s + 1) * 512]
                if use_scalar:
                    nc.scalar.copy(dst, src)
                else:
                    nc.vector.tensor_copy(dst, src)


def _emit_silu(nc, cx, tt_range):
    for tt in tt_range:
        sg = cx.p3.tile([128, DV], F16, name="sg", tag="sg", bufs=1)
        nc.scalar.activation(sg[:], cx.g_sb[:, tt * DV:(tt + 1) * DV], ACTF.Sigmoid)
        nc.vector.tensor_tensor(cx.g_sb[:, tt * DV:(tt + 1) * DV], sg[:],
                                cx.g_sb[:, tt * DV:(tt + 1) * DV], ALU.mult)


def _emit_ph4_tile(nc, cx, tt):
    """Gate, transpose, Wo projection and store for one 128-token tile."""
    xres = cx.p3.tile([128, DV], F32, name="xres", tag="xres", bufs=2)
    nc.vector.tensor_tensor(xres[:], cx.o_acc[:, tt * DV:(tt + 1) * DV],
                            cx.g_sb[:, tt * DV:(tt + 1) * DV], ALU.mult)
    tr = cx.ps.tile([128, 1024], F32, name="big", tag="big", bufs=2)
    for dvc in range(4):
        nc.tensor.matmul(tr[:, dvc * 128:(dvc + 1) * 128],
                         xres[:, dvc * 128:(dvc + 1) * 128], cx.ident,
                         is_transpose=True, start=(dvc == 0), stop=(dvc == 3))
    xtt = cx.p3.tile([128, DV], F16, name="xtt", tag="xtt", bufs=1)
    nc.vector.tensor_copy(xtt[:], tr[:, 0:DV])
    ost = cx.p3.tile([128, HID], F16, name="ost", tag="ost", bufs=2)
    for hb in range(4):
        psf = cx.ps.tile([128, 1024], F32, name="big", tag="big", bufs=2)
        for dvc in range(4):
            nc.tensor.matmul(psf[:, 0:512], xtt[:, dvc * 128:(dvc + 1) * 128],
                             cx.wg_t[dvc][:, hb * 512:(hb + 1) * 512],
                             start=(dvc == 0), stop=(dvc == 3))
        if hb % 2 == 0:
            nc.scalar.copy(ost[:, hb * 512:(hb + 1) * 512], psf[:, 0:512])
        else:
            nc.vector.tensor_copy(ost[:, hb * 512:(hb + 1) * 512], psf[:, 0:512])
    nc.sync.dma_start(out=cx.out[tt * 128:(tt + 1) * 128, :], in_=ost[:])


def _emit_attention(nc, cx, b):
    qT, kT, v_sb = cx.qT, cx.kT, cx.v_sb
    # --- kTm: shared set = plain kT slices; routed via host mask tiles ---
    ktm = [[kT[:, dc * T + b * TB:dc * T + (b + 1) * TB] for dc in range(2)]]
    for rs in range(1, 5):
        mb = cx.p3.tile([128, TB], F16, name="mb", tag="mb", bufs=1)
        nc.sync.dma_start(
            out=mb[:],
            in_=cx.mbk_d[:, ((rs - 1) * NB + b) * TB:((rs - 1) * NB + b + 1) * TB])
        pair = []
        for dc in range(2):
            kmt = cx.p3.tile([128, TB], F16, name="ktm", tag=f"ktm{rs}{dc}", bufs=1)
            for (a, ln) in QRUNS[rs + 1]:
                nc.vector.tensor_tensor(
                    kmt[:, a:a + ln],
                    kT[:, dc * T + b * TB + a:dc * T + b * TB + a + ln],
                    mb[:, a:a + ln], ALU.mult)
            pair.append(kmt)
        ktm.append(pair)
    # --- nspur_b[r', :] = -sum_{masked k} v[k, :]  (rank-4) ---
    psn = cx.ps.tile([128, 1024], F32, name="big", tag="big", bufs=2)
    for kt in range(8):
        ktt = b * 8 + kt
        nc.tensor.matmul(psn[0:NE, 0:DV], cx.nsel[:, ktt * NE:(ktt + 1) * NE],
                         v_sb[:, ktt * DV:(ktt + 1) * DV],
                         start=(kt == 0), stop=(kt == 7))
    nspur = cx.p3.tile([NE, DV], F16, name="nspur", tag="nspur", bufs=2)
    nc.scalar.copy(nspur[:], psn[0:NE, 0:DV])
    # --- qmT for all r over this batch: [r][d2c] -> [128, TB] ---
    qmT = []
    for r in range(R):
        pair = []
        for d2c in range(2):
            qm = cx.p3.tile([128, TB], F16, name="qmT", tag=f"qmT{r}{d2c}", bufs=1)
            psq = cx.ps.tile([128, 1024], F32, name="big", tag="big", bufs=2)
            qruns = QRUNS_FULL if r < 2 else QRUNS[r]
            for (a, ln) in qruns:
                for dc in range(2):
                    nc.tensor.matmul(
                        psq[:, a:a + ln],
                        cx.wqm_sb[:, dc * 1536 + r * 256 + d2c * 128:
                                  dc * 1536 + r * 256 + d2c * 128 + 128],
                        qT[:, dc * T + b * TB + a:dc * T + b * TB + a + ln],
                        start=(dc == 0), stop=(dc == 1))
            nc.vector.tensor_copy(qm[:], psq[:])
            pair.append(qm)
        qmT.append(pair)

    # --- per query-tile: scores -> exp -> combine -> AV; routed experts
    #     restricted to their fixed windows, tiles outside an expert's
    #     window skip it entirely ---
    for j in range(8):
        tt = b * 8 + j
        q0 = j * 128
        rs_list = [0, 1] + TILE_EXPERTS[j]
        seq = []
        for r in rs_list:
            seq += [(r, gc) for gc in (range(8) if r < 2 else WIN[r])]
        last_per_bank = {}
        for si, (r_, gc_) in enumerate(seq):
            last_per_bank[gc_ // 4] = si
        ptps = cx.ps.tile([128, 1024], F32, name="acc", tag="acc", bufs=2)
        cmul16 = cx.p3.tile([128, R], F16, name="cmul16", tag="cmula", bufs=2)
        nc.vector.memset(cmul16[:], 0.0)
        si = 0
        for r in rs_list:
            krs = 0 if r < 2 else r - 1
            runs = RUNS_FULL if r < 2 else RUNS[r]
            chunks = list(range(8)) if r < 2 else list(WIN[r])
            width = 128 * len(chunks)
            sps = cx.ps.tile([128, 1024], F32, name="big", tag="big", bufs=2)
            for (pc, gc0, nch) in runs:
                for d2c in range(2):
                    nc.tensor.matmul(
                        sps[:, pc * 128:pc * 128 + nch * 128],
                        qmT[r][d2c][:, q0:q0 + 128],
                        ktm[krs][d2c][:, gc0 * 128:gc0 * 128 + nch * 128],
                        start=(d2c == 0), stop=(d2c == 1))
            es = cx.p3.tile([128, 1024], F16, name="es", tag="es", bufs=2)
            dn = cx.p3.tile([128, 1], F32, name="dn", tag="dn", bufs=4)
            nc.scalar.activation(es[:, 0:width], sps[:, 0:width], ACTF.Exp,
                                 scale=SCALE, accum_out=dn[:])
            dinv = cx.p3.tile([128, 1], F32, name="adinv", tag="adinv", bufs=4)
            if r < 2:
                nc.vector.reciprocal(dinv[:], dn[:])
            else:
                dnc = cx.p3.tile([128, 1], F32, name="dnc", tag="dnc", bufs=4)
                nc.vector.tensor_scalar(dnc[:], dn[:], NCOUT, None, ALU.add)
                nc.vector.reciprocal(dinv[:], dnc[:])
            dcd = cx.p3.tile([128, 128], F16, name="dcd", tag="dcd", bufs=2)
            nc.vector.tensor_scalar(dcd[:], cx.ident16[:], dinv[:],
                                    cx.rw_all[:, tt * R + r:tt * R + r + 1],
                                    ALU.mult, ALU.mult)
            nc.vector.tensor_tensor(cmul16[:, r:r + 1], dinv[:],
                                    cx.rw_all[:, tt * R + r:tt * R + r + 1],
                                    ALU.mult)
            # combine: start only on the first matmul touching each
            # PSUM bank (start clears has_written bank-wide).
            for pi, gc in enumerate(chunks):
                nc.tensor.matmul(
                    ptps[:, gc * 128:(gc + 1) * 128],
                    es[:, pi * 128:(pi + 1) * 128],
                    dcd[:], start=(r == 0 and gc % 4 == 0),
                    stop=(si == last_per_bank[gc // 4]))
                si += 1
        pts = cx.p3.tile([128, 1024], F16, name="pts", tag="pts", bufs=2)
        nc.scalar.copy(pts[:], ptps[:])
        # CT: [4, 128] = cmul16[:, 2:6]^T (f16 transpose)
        psc = cx.ps.tile([128, 1024], F32, name="acc", tag="acc", bufs=2)
        nc.tensor.matmul(psc[0:NE, 0:128], cmul16[:, 2:R], cx.ident16[:],
                         start=True, stop=True)
        ctb = cx.p3.tile([NE, 128], F16, name="ctb", tag="ctb", bufs=2)
        nc.scalar.copy(ctb[:], psc[0:NE, 0:128])
        # AV + spur correction
        avp = cx.ps.tile([128, 1024], F32, name="acc", tag="acc", bufs=2)
        for kt in range(8):
            ktt = b * 8 + kt
            nc.tensor.matmul(avp[:, 0:DV], pts[:, kt * 128:(kt + 1) * 128],
                             v_sb[:, ktt * DV:(ktt + 1) * DV],
                             start=(kt == 0), stop=False)
        nc.tensor.matmul(avp[:, 0:DV], ctb[:], nspur[:], start=False, stop=True)
        nc.vector.tensor_copy(cx.o_acc[:, tt * DV:(tt + 1) * DV], avp[:, 0:DV])


def _body(ctx, nc, tc, io):
    wq, wk, wv, wg, wqm, hst_d, rw_d, nsel_d, mbk_d, wo, out = io

    cx = Ctx()
    cx.out = out
    cx.mbk_d = mbk_d
    const = ctx.enter_context(tc.tile_pool(name="const", bufs=1))
    pers = ctx.enter_context(tc.tile_pool(name="pers", bufs=1))

    from concourse.masks import make_identity
    cx.ident = const.tile([128, 128], F32, name="ident")
    make_identity(nc, cx.ident)
    cx.ident16 = const.tile([128, 128], F16, name="ident16")
    nc.vector.tensor_copy(cx.ident16[:], cx.ident[:])

    cx.qT = pers.tile([128, 2 * T], F16, name="qT")         # [d-chunk, token]
    cx.kT = pers.tile([128, 2 * T], F16, name="kT")         # [d-chunk, token]
    cx.v_sb = pers.tile([128, 16 * DV], F16, name="v_sb")   # [token-tile, dv]
    cx.g_sb = pers.tile([128, 16 * DV], F16, name="g_sb")   # [token-tile, dv]
    cx.wqm_sb = pers.tile([128, 2 * 1536], F16, name="wqm_sb")
    cx.rw_all = pers.tile([128, 16 * R], F32, name="rw_all")
    cx.nsel = pers.tile([128, 16 * NE], F16, name="nsel")   # sel - 1 (0/-1)
    cx.o_acc = pers.tile([128, 16 * DV], F16, name="o_acc")

    with tc.tile_pool(name="p1w", bufs=1) as p1w, \
         tc.tile_pool(name="p1", bufs=1) as p1, \
         tc.tile_pool(name="p3", bufs=1) as p3, \
         tc.tile_pool(name="ps", bufs=1, space="PSUM") as ps_pool:
        cx.p3 = p3
        cx.ps = ps_pool

        def hst_dma(tb, i):
            h3 = p1.tile([128, 1024], F16, name="hst", tag="hst", bufs=10)
            nc.sync.dma_start(
                out=h3[:].rearrange("p (c t) -> p c t", c=2),
                in_=hst_d[i * 256:(i + 1) * 256,
                          tb * 512:tb * 512 + 512].rearrange(
                    "(c p) t -> p c t", p=128))
            return h3

        def wdma(src, i, nchunk, nm):
            w1 = p1w.tile([128, 2048], F16, name=nm, tag=f"{nm}{i}")
            nc.sync.dma_start(
                out=w1[:].rearrange("p (c d) -> p c d", c=nchunk),
                in_=src[i * (128 * nchunk):(i + 1) * (128 * nchunk), :].rearrange(
                    "(c p) d -> p c d", p=128))
            return w1

        # DMA stream paced to the q/k chain: weights slotted where the
        # consuming chain is still ahead of the transfer clock.
        cx.wq_t, cx.wk_t, cx.wv_t, cx.wg_t = [], [], [], []
        hst0 = [hst_dma(0, 0)]
        cx.wq_t.append(wdma(wq, 0, 8, "wqt"))
        for i in range(1, 4):
            hst0.append(hst_dma(0, i))
        cx.wq_t.append(wdma(wq, 1, 8, "wqt"))
        for i in range(4, 8):
            hst0.append(hst_dma(0, i))
        cx.wk_t.append(wdma(wk, 0, 8, "wkt"))
        cx.wk_t.append(wdma(wk, 1, 8, "wkt"))
        for i in range(4):
            cx.wv_t.append(wdma(wv, i, 4, "wvt"))
        for i in range(4):
            cx.wg_t.append(wdma(wg, i, 4, "wgt"))
        hst1 = [hst_dma(1, i) for i in range(8)]
        nc.sync.dma_start(out=cx.rw_all[:], in_=rw_d[:, :])
        nc.sync.dma_start(out=cx.nsel[:], in_=nsel_d[:, :])
        for dc in range(2):
            nc.sync.dma_start(out=cx.wqm_sb[:, dc * 1536:(dc + 1) * 1536],
                              in_=wqm[dc * 128:(dc + 1) * 128, :])

        _emit_phase1_tb(nc, cx, 0, hst0)
        _emit_phase1_tb(nc, cx, 1, hst1)
        _emit_silu(nc, cx, range(0, 8))
        _emit_attention(nc, cx, 0)
        hst2 = [hst_dma(2, i) for i in range(8)]
        _emit_phase1_tb(nc, cx, 2, hst2)
        hst3 = [hst_dma(3, i) for i in range(8)]
        _emit_phase1_tb(nc, cx, 3, hst3)
        _emit_silu(nc, cx, range(8, 16))
        # wg tiles are dead after tb3's g chains: recycle them for Wo
        # (prefetches during attention(b1) at zero SBUF cost).
        for i in range(4):
            nc.sync.dma_start(out=cx.wg_t[i][:], in_=wo[i * 128:(i + 1) * 128, :])
        _emit_attention(nc, cx, 1)
        for tt in range(16):
            _emit_ph4_tile(nc, cx, tt)


_PROGRAM = None


def build_program():
    global _PROGRAM
    if _PROGRAM is not None:
        return _PROGRAM
    nc = bacc.Bacc("TRN2", target_bir_lowering=False, debug=False, num_devices=8)
    names = [("wq", [HID, D], F16), ("wk", [HID, D], F16),
             ("wv", [HID, DV], F16), ("wg", [HID, DV], F16),
             ("wqm", [D, D * R], F16),
             ("hst", [HID, T], F16),
             ("rw", [128, 16 * R], F32), ("nsel", [128, 16 * NE], F16),
             ("mbk", [128, NE * NB * TB], F16), ("wo", [DV, HID], F16)]
    io = [nc.dram_tensor(n, s, dt, kind="ExternalInput").ap() for n, s, dt in names]
    io.append(nc.dram_tensor("out", [T, HID], F16, kind="ExternalOutput").ap())
    with tile.TileContext(nc) as tc:
        from contextlib import ExitStack as ES
        with ES() as ctx:
            _body(ctx, nc, tc, io)
    nc.compile()
    _PROGRAM = nc
    return nc


def _host_routing(hs64, Wq64, Wgate64):
    """Per-head routing on host, matching the reference in f64.

    Returns rw [T, R] f32, msel [T, NE] f16 (1 active / 0 masked)."""
    S, K = 2, 2
    T_, _ = hs64.shape
    rw = np.zeros((T_, R), np.float64)
    rw[:, :S] = 0.25
    logits = hs64 @ (Wq64 @ Wgate64)          # [T, R-S]
    sc = np.exp(logits - logits.max(axis=-1, keepdims=True))
    sc /= sc.sum(axis=-1, keepdims=True)
    idx = np.argsort(-sc, axis=-1, kind="stable")[:, :K]   # top-2, ties -> low idx
    w = np.take_along_axis(sc, idx, axis=-1)
    w /= w.sum(axis=-1, keepdims=True)
    np.put_along_axis(rw[:, S:], idx, w * 0.5, axis=-1)
    msel = np.zeros((T_, NE), np.float64)
    np.put_along_axis(msel, idx, 1.0, axis=-1)
    return rw.astype(np.float32), msel.astype(np.float16)


def _assign_chunks(msel_b):
    """Assign each of the 1024 tokens to a 128-chunk such that both of its
    selected experts' windows contain the chunk.  Exact integer max-flow on
    the 6-pair x 4-super-chunk transportation graph (always feasible for
    the ~Binomial(1024, 1/6) group sizes this routing produces).

    Returns perm [1024]: position -> original token index."""
    sel = msel_b.astype(bool)
    pair_of = np.zeros(TB, np.int64)
    for p, (i_, j_) in enumerate(PAIRS):
        pair_of[sel[:, i_ - 2] & sel[:, j_ - 2]] = p
    n = np.bincount(pair_of, minlength=6)
    # Edmonds-Karp max-flow: node 0 = source, 1..6 = pairs, 7..10 = super-
    # chunks, 11 = sink.  source->pair cap n[p]; pair->sc cap inf; sc->sink 256.
    NV = 12
    cap = np.zeros((NV, NV), np.int64)
    for p in range(6):
        cap[0, 1 + p] = n[p]
        for s in REGION[p]:
            cap[1 + p, 7 + s] = TB
    for s in range(4):
        cap[7 + s, 11] = TB // 4
    total = 0
    while True:
        # BFS for augmenting path
        par = [-1] * NV
        par[0] = 0
        queue = [0]
        while queue:
            u = queue.pop(0)
            for v in range(NV):
                if par[v] < 0 and cap[u, v] > 0:
                    par[v] = u
                    queue.append(v)
        if par[11] < 0:
            break
        # min residual along path
        v, aug = 11, 1 << 40
        while v != 0:
            u = par[v]
            aug = min(aug, cap[u, v])
            v = u
        v = 11
        while v != 0:
            u = par[v]
            cap[u, v] -= aug
            cap[v, u] += aug
            v = u
        total += aug
    assert total == TB, f"chunk assignment infeasible (flow {total})"
    x = np.zeros((6, 4), np.int64)
    for p in range(6):
        for s in REGION[p]:
            x[p, s] = cap[7 + s, 1 + p]   # reverse edge = flow
    # build permutation: tokens of pair p fill super-chunks per x[p, :]
    sc_members = [[] for _ in range(4)]
    for p in range(6):
        toks = np.nonzero(pair_of == p)[0]
        o = 0
        for s in range(4):
            sc_members[s].extend(toks[o:o + x[p, s]])
            o += x[p, s]
    perm = np.concatenate([np.asarray(sc_members[s], np.int64) for s in range(4)])
    assert perm.shape[0] == TB
    return perm


def make_in_maps(hidden_states, Wq, Wk, Wv, Wq_exp, Wk_exp, Wgate, Wg, Wo):
    hs2 = np.asarray(hidden_states, np.float32).reshape(T, HID)
    hsT = np.ascontiguousarray(hs2.T.astype(np.float16))
    hs64 = hs2.astype(np.float64)
    Wq64 = np.asarray(Wq, np.float64)
    Wg64 = np.asarray(Wgate, np.float64)
    Wqe64 = np.asarray(Wq_exp, np.float64)
    Wke64 = np.asarray(Wk_exp, np.float64)
    in_maps = []
    perms = []
    # chunk index -> window membership per routed expert, for nsel'
    in_win = np.zeros((8, NE), bool)
    for r in WIN:
        for gc in WIN[r]:
            in_win[gc, r - 2] = True
    for c in range(8):
        rw, msel = _host_routing(hs64, Wq64[:, c * D:(c + 1) * D], Wg64)
        perm = np.empty(T, np.int64)
        for b in range(NB):
            pb = _assign_chunks(msel[b * TB:(b + 1) * TB])
            perm[b * TB:(b + 1) * TB] = b * TB + pb
        perms.append(perm)
        rw_p = rw[perm]
        msel_p = msel[perm].astype(np.float32)
        # nsel': -1 only for masked keys inside the expert's window
        chunk_of = (np.arange(T) % TB) // 128   # chunk within batch
        nsel_p = (msel_p - 1) * in_win[chunk_of, :]
        rw_sb = np.ascontiguousarray(
            rw_p.reshape(16, 128, R).transpose(1, 0, 2).reshape(128, 16 * R))
        nsel_sb = np.ascontiguousarray(
            nsel_p.reshape(16, 128, NE).transpose(1, 0, 2).reshape(128, 16 * NE)
        ).astype(np.float16)
        mbk = np.empty((128, NE * NB * TB), np.float16)
        for i in range(NE):
            for b in range(NB):
                mbk[:, (i * NB + b) * TB:(i * NB + b + 1) * TB] = \
                    msel_p[b * TB:(b + 1) * TB, i][None, :]
        wqm = np.empty((D, D * R), np.float16)
        for r in range(R):
            m = Wqe64[c][:, r * D:(r + 1) * D] @ Wke64[c][:, r * D:(r + 1) * D].T
            wqm[:, r * D:(r + 1) * D] = m.astype(np.float16)
        in_maps.append({
            "wq": np.asarray(Wq, np.float16)[:, c * D:(c + 1) * D].copy(),
            "wk": np.asarray(Wk, np.float16)[:, c * D:(c + 1) * D].copy(),
            "wv": np.asarray(Wv, np.float16)[:, c * DV:(c + 1) * DV].copy(),
            "wg": np.asarray(Wg, np.float16)[:, c * DV:(c + 1) * DV].copy(),
            "wqm": wqm,
            "hst": np.ascontiguousarray(hsT[:, perm]),
            "rw": rw_sb, "nsel": nsel_sb, "mbk": mbk,
            "wo": np.asarray(Wo, np.float16)[c * DV:(c + 1) * DV, :].copy(),
        })
    return in_maps, perms


def kernel(hidden_states, Wq, Wk, Wv, Wq_exp, Wk_exp, Wgate, Wg, Wo):
    nc = build_program()
    in_maps, perms = make_in_maps(hidden_states, Wq, Wk, Wv, Wq_exp, Wk_exp,
                                  Wgate, Wg, Wo)
    res = run_bass_kernel_spmd(nc, in_maps, list(range(8))).results
    out = np.zeros((T, HID), np.float32)
    for c in range(8):
        out[perms[c]] += res[c]["out"].astype(np.float32)
    return out.reshape(2, 1024, HID).astype(np.float32)


# revision 33
# speedup vs baseline: 1.1920x; 1.1920x over previous
"""Trainium2 Bass kernel for MockMobGatedDeltaNetMoE (v17, ~453us from 802us).

Sharding: head-parallel over H=8 heads, one head per NeuronCore.
Each core computes its head's full contribution; the host sums the 8
partial output projections (per-core token permutations undone on host).

Key design points (history: v8 802us -> v9 552 -> v11 495 -> v15 457 -> v17 453):
 - Routing (softmax top-2 over the 4 routed experts) runs on the HOST in
   f64 from logits = hs @ (Wq_head @ Wgate); the device receives combine
   weights rw, spur-correction selectors nsel, and broadcast key masks.
   This removes 768 LDWEIGHTS-bound tiny matmuls (~75us) and halves the
   hidden-state DMA (single f16 tensor).
 - Fixed expert windows: each routed expert owns a constant 6-of-8
   key-chunk window (identical across cores, so one SPMD program).  The
   host solves a tiny exact max-flow per (core, batch) assigning every
   token to a 128-chunk inside BOTH of its selected experts' windows,
   then permutes tokens accordingly.  Scores/exp/combine for a routed
   expert only touch its window (packed PSUM layout); the 256 keys
   outside the window are all masked and contribute exactly exp(0)=1
   each to the softmax denominator (+256 constant).  Tiles outside an
   expert's window skip that expert entirely (5 of 6 experts per tile).
 - Score fusion: S_r = q @ (Wq_exp_r @ Wk_exp_r^T) @ k^T with M_r fused
   on host; masked keys give exp(0)=1 (reference semantics) and their
   masked-v contribution is removed by a rank-4 spur correction.
 - One merged exp per (query-tile, expert) with accum_out denominator;
   expert combine via diagonal matmuls (transpose+scale+accumulate in
   PSUM); one attention @ V matmul per query tile.
 - Emission is batch-interleaved (tb0,tb1 -> attn b0 -> tb2,tb3 ->
   attn b1 -> phase 4) over one shared 8-bank PSUM scheme; DMAs ship as
   multi-chunk [128, 2048] tiles via 3D access patterns (~0.6us issue
   cost each); Wo is DMA'd into the dead wg weight tiles after the last
   g-projection; one [128, 2048] store + one DMA per output tile.
"""

import numpy as np

import concourse.bass as bass
import concourse.bacc as bacc
import concourse.tile as tile
from concourse import mybir
from concourse.bass_utils import run_bass_kernel_spmd

F32 = mybir.dt.float32
F16 = mybir.dt.float16
ALU = mybir.AluOpType
ACTF = mybir.ActivationFunctionType

H, D, R, NE = 8, 256, 6, 4
HID, DV, T = 2048, 512, 2048
NB = 2
TB = T // NB
SCALE = 1.0 / 16.0

# ---- fixed expert-window scheme (uniform across cores -> one SPMD program).
# Each routed expert r owns a fixed 6-of-8 chunk window; the host permutes
# tokens (per core/batch) so every token lands in a chunk inside both of its
# selected experts' windows.  Keys outside W_r are all masked for r and
# contribute exactly exp(0)=1 each to the softmax denominator (+256 const).
WIN = {2: (0, 1, 2, 3, 4, 5), 3: (2, 3, 4, 5, 6, 7),
       4: (0, 1, 2, 3, 6, 7), 5: (0, 1, 4, 5, 6, 7)}
# score-matmul runs per routed expert: (packed_chunk_start, global_chunk_start, nchunks)
RUNS = {2: ((0, 0, 4), (4, 4, 2)), 3: ((0, 2, 4), (4, 6, 2)),
        4: ((0, 0, 4), (4, 6, 2)), 5: ((0, 0, 2), (2, 4, 2), (4, 6, 2))}
RUNS_FULL = ((0, 0, 4), (4, 4, 4))
# token runs (start, len) per routed expert's window, split at 512 boundaries
QRUNS = {2: ((0, 512), (512, 256)), 3: ((256, 256), (512, 512)),
         4: ((0, 512), (768, 256)), 5: ((0, 256), (512, 512))}
QRUNS_FULL = ((0, 512), (512, 512))
TILE_EXPERTS = [sorted(r for r in WIN if j in WIN[r]) for j in range(8)]
PAIRS = [(2, 3), (2, 4), (2, 5), (3, 4), (3, 5), (4, 5)]
# super-chunks (pairs of 128-chunks) allowed per expert pair
REGION = [(1, 2), (0, 1), (0, 2), (1, 3), (2, 3), (0, 3)]
NCOUT = 256.0   # keys outside a routed expert's window (all masked): 2 chunks


class Ctx:
    pass


def _emit_phase1_tb(nc, cx, tb, hst_t):
    """q/k/v/g projection chains for one 512-token block.

    hst_t: 8 tiles [128, 1024], chunk hc at hst_t[hc//2][:, (hc%2)*512:...]."""
    t0 = tb * 512

    def hst_mv(hc):
        return hst_t[hc // 2][:, (hc % 2) * 512:(hc % 2) * 512 + 512]

    # q/k -> transposed [d-chunk, token]; f0+f1 share one 2-bank psum
    for wt, dstT in ((cx.wq_t, cx.qT), (cx.wk_t, cx.kT)):
        ps = cx.ps.tile([128, 1024], F32, name="big", tag="big", bufs=2)
        for hc in range(16):
            wsl = wt[hc // 8]
            c0 = (hc % 8) * 256
            nc.tensor.matmul(ps[:, 0:512], wsl[:, c0:c0 + 128], hst_mv(hc),
                             start=(hc == 0), stop=(hc == 15))
            nc.tensor.matmul(ps[:, 512:1024], wsl[:, c0 + 128:c0 + 256], hst_mv(hc),
                             start=(hc == 0), stop=(hc == 15))
        nc.scalar.copy(dstT[:, t0:t0 + 512], ps[:, 0:512])
        nc.scalar.copy(dstT[:, T + t0:T + t0 + 512], ps[:, 512:1024])
    # v then g (separate passes; wg arrives after wv in the DMA stream)
    for wt, dst_sb, use_scalar in ((cx.wv_t, cx.v_sb, True), (cx.wg_t, cx.g_sb, False)):
        for half in range(2):
            ps = cx.ps.tile([128, 1024], F32, name="big", tag="big", bufs=2)
            for hc in range(16):
                wmv = wt[hc // 4][:, (hc % 4) * 512:(hc % 4) * 512 + 512]
                h0 = (hc % 2) * 512 + half * 256
                nc.tensor.matmul(ps[:, 0:512], hst_t[hc // 2][:, h0:h0 + 128],
                                 wmv, start=(hc == 0), stop=(hc == 15))
                nc.tensor.matmul(ps[:, 512:1024], hst_t[hc // 2][:, h0 + 128:h0 + 256],
                                 wmv, start=(hc == 0), stop=(hc == 15))
            tt = tb * 4 + half * 2
            for s in range(2):
                dst = dst_sb[:, (tt + s) * DV:(tt + s + 1) * DV]
                src = ps[:, s * 512:(s + 1) * 512]
                if use_scalar:
                    nc.scalar.copy(dst, src)
                else:
                    nc.vector.tensor_copy(dst, src)


def _emit_silu(nc, cx, tt_range):
    for tt in tt_range:
        sg = cx.p3.tile([128, DV], F16, name="sg", tag="sg", bufs=1)
        nc.scalar.activation(sg[:], cx.g_sb[:, tt * DV:(tt + 1) * DV], ACTF.Sigmoid)
        nc.vector.tensor_tensor(cx.g_sb[:, tt * DV:(tt + 1) * DV], sg[:],
                                cx.g_sb[:, tt * DV:(tt + 1) * DV], ALU.mult)


def _emit_ph4_tile(nc, cx, tt):
    """Gate, transpose, Wo projection and store for one 128-token tile."""
    xres = cx.p3.tile([128, DV], F32, name="xres", tag="xres", bufs=2)
    nc.vector.tensor_tensor(xres[:], cx.o_acc[:, tt * DV:(tt + 1) * DV],
                            cx.g_sb[:, tt * DV:(tt + 1) * DV], ALU.mult)
    tr = cx.ps.tile([128, 1024], F32, name="big", tag="big", bufs=2)
    for dvc in range(4):
        nc.tensor.matmul(tr[:, dvc * 128:(dvc + 1) * 128],
                         xres[:, dvc * 128:(dvc + 1) * 128], cx.ident,
                         is_transpose=True, start=(dvc == 0), stop=(dvc == 3))
    xtt = cx.p3.tile([128, DV], F16, name="xtt", tag="xtt", bufs=1)
    nc.vector.tensor_copy(xtt[:], tr[:, 0:DV])
    ost = cx.p3.tile([128, HID], F16, name="ost", tag="ost", bufs=2)
    for hb in range(4):
        psf = cx.ps.tile([128, 1024], F32, name="big", tag="big", bufs=2)
        for dvc in range(4):
            nc.tensor.matmul(psf[:, 0:512], xtt[:, dvc * 128:(dvc + 1) * 128],
                             cx.wg_t[dvc][:, hb * 512:(hb + 1) * 512],
                             start=(dvc == 0), stop=(dvc == 3))
        if hb % 2 == 0:
            nc.scalar.copy(ost[:, hb * 512:(hb + 1) * 512], psf[:, 0:512])
        else:
            nc.vector.tensor_copy(ost[:, hb * 512:(hb + 1) * 512], psf[:, 0:512])
    nc.sync.dma_start(out=cx.out[tt * 128:(tt + 1) * 128, :], in_=ost[:])


def _emit_attention(nc, cx, b):
    qT, kT, v_sb = cx.qT, cx.kT, cx.v_sb
    # --- kTm: shared set = plain kT slices; routed via host mask tiles ---
    ktm = [[kT[:, dc * T + b * TB:dc * T + (b + 1) * TB] for dc in range(2)]]
    for rs in range(1, 5):
        mb = cx.p3.tile([128, TB], F16, name="mb", tag="mb", bufs=1)
        nc.sync.dma_start(
            out=mb[:],
            in_=cx.mbk_d[:, ((rs - 1) * NB + b) * TB:((rs - 1) * NB + b + 1) * TB])
        pair = []
        for dc in range(2):
            kmt = cx.p3.tile([128, TB], F16, name="ktm", tag=f"ktm{rs}{dc}", bufs=1)
            for (a, ln) in QRUNS[rs + 1]:
                nc.vector.tensor_tensor(
                    kmt[:, a:a + ln],
                    kT[:, dc * T + b * TB + a:dc * T + b * TB + a + ln],
                    mb[:, a:a + ln], ALU.mult)
            pair.append(kmt)
        ktm.append(pair)
    # --- nspur_b[r', :] = -sum_{masked k} v[k, :]  (rank-4) ---
    psn = cx.ps.tile([128, 1024], F32, name="big", tag="big", bufs=2)
    for kt in range(8):
        ktt = b * 8 + kt
        nc.tensor.matmul(psn[0:NE, 0:DV], cx.nsel[:, ktt * NE:(ktt + 1) * NE],
                         v_sb[:, ktt * DV:(ktt + 1) * DV],
                         start=(kt == 0), stop=(kt == 7))
    nspur = cx.p3.tile([NE, DV], F16, name="nspur", tag="nspur", bufs=2)
    nc.scalar.copy(nspur[:], psn[0:NE, 0:DV])
    # --- qmT for all r over this batch: [r][d2c] -> [128, TB] ---
    qmT = []
    for r in range(R):
        pair = []
        for d2c in range(2):
            qm = cx.p3.tile([128, TB], F16, name="qmT", tag=f"qmT{r}{d2c}", bufs=1)
            psq = cx.ps.tile([128, 1024], F32, name="big", tag="big", bufs=2)
            qruns = QRUNS_FULL if r < 2 else QRUNS[r]
            for (a, ln) in qruns:
                for dc in range(2):
                    nc.tensor.matmul(
                        psq[:, a:a + ln],
                        cx.wqm_sb[:, dc * 1536 + r * 256 + d2c * 128:
                                  dc * 1536 + r * 256 + d2c * 128 + 128],
                        qT[:, dc * T + b * TB + a:dc * T + b * TB + a + ln],
                        start=(dc == 0), stop=(dc == 1))
            nc.vector.tensor_copy(qm[:], psq[:])
            pair.append(qm)
        qmT.append(pair)

    # --- per query-tile: scores -> exp -> combine -> AV; routed experts
    #     restricted to their fixed windows, tiles outside an expert's
    #     window skip it entirely ---
    for j in range(8):
        tt = b * 8 + j
        q0 = j * 128
        rs_list = [0, 1] + TILE_EXPERTS[j]
        seq = []
        for r in rs_list:
            seq += [(r, gc) for gc in (range(8) if r < 2 else WIN[r])]
        last_per_bank = {}
        for si, (r_, gc_) in enumerate(seq):
            last_per_bank[gc_ // 4] = si
        ptps = cx.ps.tile([128, 1024], F32, name="acc", tag="acc", bufs=2)
        cmul16 = cx.p3.tile([128, R], F16, name="cmul16", tag="cmula", bufs=2)
        nc.vector.memset(cmul16[:], 0.0)
        si = 0
        for r in rs_list:
            krs = 0 if r < 2 else r - 1
            runs = RUNS_FULL if r < 2 else RUNS[r]
            chunks = list(range(8)) if r < 2 else list(WIN[r])
            width = 128 * len(chunks)
            sps = cx.ps.tile([128, 1024], F32, name="big", tag="big", bufs=2)
            for (pc, gc0, nch) in runs:
                for d2c in range(2):
                    nc.tensor.matmul(
                        sps[:, pc * 128:pc * 128 + nch * 128],
                        qmT[r][d2c][:, q0:q0 + 128],
                        ktm[krs][d2c][:, gc0 * 128:gc0 * 128 + nch * 128],
                        start=(d2c == 0), stop=(d2c == 1))
            es = cx.p3.tile([128, 1024], F16, name="es", tag="es", bufs=2)
            dn = cx.p3.tile([128, 1], F32, name="dn", tag="dn", bufs=4)
            nc.scalar.activation(es[:, 0:width], sps[:, 0:width], ACTF.Exp,
                                 scale=SCALE, accum_out=dn[:])
            dinv = cx.p3.tile([128, 1], F32, name="adinv", tag="adinv", bufs=4)
            if r < 2:
                nc.vector.reciprocal(dinv[:], dn[:])
            else:
                dnc = cx.p3.tile([128, 1], F32, name="dnc", tag="dnc", bufs=4)
                nc.vector.tensor_scalar(dnc[:], dn[:], NCOUT, None, ALU.add)
                nc.vector.reciprocal(dinv[:], dnc[:])
            dcd = cx.p3.tile([128, 128], F16, name="dcd", tag="dcd", bufs=2)
            nc.vector.tensor_scalar(dcd[:], cx.ident16[:], dinv[:],
                                    cx.rw_all[:, tt * R + r:tt * R + r + 1],
                                    ALU.mult, ALU.mult)
            nc.vector.tensor_tensor(cmul16[:, r:r + 1], dinv[:],
                                    cx.rw_all[:, tt * R + r:tt * R + r + 1],
                                    ALU.mult)
            # combine: start only on the first matmul touching each
            # PSUM bank (start clears has_written bank-wide).
            for pi, gc in enumerate(chunks):
                nc.tensor.matmul(
                    ptps[:, gc * 128:(gc + 1) * 128],
                    es[:, pi * 128:(pi + 1) * 128],
                    dcd[:], start=(r == 0 and gc % 4 == 0),
                    stop=(si == last_per_bank[gc // 4]))
                si += 1
        pts = cx.p3.tile([128, 1024], F16, name="pts", tag="pts", bufs=2)
        nc.scalar.copy(pts[:], ptps[:])
        # CT: [4, 128] = cmul16[:, 2:6]^T (f16 transpose)
        psc = cx.ps.tile([128, 1024], F32, name="acc", tag="acc", bufs=2)
        nc.tensor.matmul(psc[0:NE, 0:128], cmul16[:, 2:R], cx.ident16[:],
                         start=True, stop=True)
        ctb = cx.p3.tile([NE, 128], F16, name="ctb", tag="ctb", bufs=2)
        nc.scalar.copy(ctb[:], psc[0:NE, 0:128])
        # AV + spur correction
        avp = cx.ps.tile([128, 1024], F32, name="acc", tag="acc", bufs=2)
        for kt in range(8):
            ktt = b * 8 + kt
            nc.tensor.matmul(avp[:, 0:DV], pts[:, kt * 128:(kt + 1) * 128],
                             v_sb[:, ktt * DV:(ktt + 1) * DV],
                             start=(kt == 0), stop=False)
        nc.tensor.matmul(avp[:, 0:DV], ctb[:], nspur[:], start=False, stop=True)
        nc.vector.tensor_copy(cx.o_acc[:, tt * DV:(tt + 1) * DV], avp[:, 0:DV])


def _body(ctx, nc, tc, io):
    wq, wk, wv, wg, wqm, hst_d, rw_d, nsel_d, mbk_d, wo, out = io

    cx = Ctx()
    cx.out = out
    cx.mbk_d = mbk_d
    const = ctx.enter_context(tc.tile_pool(name="const", bufs=1))
    pers = ctx.enter_context(tc.tile_pool(name="pers", bufs=1))

    from concourse.masks import make_identity
    cx.ident = const.tile([128, 128], F32, name="ident")
    make_identity(nc, cx.ident)
    cx.ident16 = const.tile([128, 128], F16, name="ident16")
    nc.vector.tensor_copy(cx.ident16[:], cx.ident[:])

    cx.qT = pers.tile([128, 2 * T], F16, name="qT")         # [d-chunk, token]
    cx.kT = pers.tile([128, 2 * T], F16, name="kT")         # [d-chunk, token]
    cx.v_sb = pers.tile([128, 16 * DV], F16, name="v_sb")   # [token-tile, dv]
    cx.g_sb = pers.tile([128, 16 * DV], F16, name="g_sb")   # [token-tile, dv]
    cx.wqm_sb = pers.tile([128, 2 * 1536], F16, name="wqm_sb")
    cx.rw_all = pers.tile([128, 16 * R], F32, name="rw_all")
    cx.nsel = pers.tile([128, 16 * NE], F16, name="nsel")   # sel - 1 (0/-1)
    cx.o_acc = pers.tile([128, 16 * DV], F16, name="o_acc")

    with tc.tile_pool(name="p1w", bufs=1) as p1w, \
         tc.tile_pool(name="p1", bufs=1) as p1, \
         tc.tile_pool(name="p3", bufs=1) as p3, \
         tc.tile_pool(name="ps", bufs=1, space="PSUM") as ps_pool:
        cx.p3 = p3
        cx.ps = ps_pool

        def hst_dma(tb, i):
            h3 = p1.tile([128, 1024], F16, name="hst", tag="hst", bufs=10)
            nc.sync.dma_start(
                out=h3[:].rearrange("p (c t) -> p c t", c=2),
                in_=hst_d[i * 256:(i + 1) * 256,
                          tb * 512:tb * 512 + 512].rearrange(
                    "(c p) t -> p c t", p=128))
            return h3

        def wdma(src, i, nchunk, nm):
            w1 = p1w.tile([128, 2048], F16, name=nm, tag=f"{nm}{i}")
            nc.sync.dma_start(
                out=w1[:].rearrange("p (c d) -> p c d", c=nchunk),
                in_=src[i * (128 * nchunk):(i + 1) * (128 * nchunk), :].rearrange(
                    "(c p) d -> p c d", p=128))
            return w1

        # DMA stream paced to the q/k chain: weights slotted where the
        # consuming chain is still ahead of the transfer clock.
        cx.wq_t, cx.wk_t, cx.wv_t, cx.wg_t = [], [], [], []
        hst0 = [hst_dma(0, 0)]
        cx.wq_t.append(wdma(wq, 0, 8, "wqt"))
        for i in range(1, 4):
            hst0.append(hst_dma(0, i))
        cx.wq_t.append(wdma(wq, 1, 8, "wqt"))
        for i in range(4, 6):
            hst0.append(hst_dma(0, i))
        cx.wk_t.append(wdma(wk, 0, 8, "wkt"))
        for i in range(6, 8):
            hst0.append(hst_dma(0, i))
        cx.wk_t.append(wdma(wk, 1, 8, "wkt"))
        for i in range(4):
            cx.wv_t.append(wdma(wv, i, 4, "wvt"))
        for i in range(4):
            cx.wg_t.append(wdma(wg, i, 4, "wgt"))
        hst1 = [hst_dma(1, i) for i in range(8)]
        nc.sync.dma_start(out=cx.rw_all[:], in_=rw_d[:, :])
        nc.sync.dma_start(out=cx.nsel[:], in_=nsel_d[:, :])
        for dc in range(2):
            nc.sync.dma_start(out=cx.wqm_sb[:, dc * 1536:(dc + 1) * 1536],
                              in_=wqm[dc * 128:(dc + 1) * 128, :])

        _emit_phase1_tb(nc, cx, 0, hst0)
        _emit_phase1_tb(nc, cx, 1, hst1)
        _emit_silu(nc, cx, range(0, 8))
        _emit_attention(nc, cx, 0)
        hst2 = [hst_dma(2, i) for i in range(8)]
        _emit_phase1_tb(nc, cx, 2, hst2)
        hst3 = [hst_dma(3, i) for i in range(8)]
        _emit_phase1_tb(nc, cx, 3, hst3)
        _emit_silu(nc, cx, range(8, 16))
        # wg tiles are dead after tb3's g chains: recycle them for Wo
        # (prefetches during attention(b1) at zero SBUF cost).
        for i in range(4):
            nc.sync.dma_start(out=cx.wg_t[i][:], in_=wo[i * 128:(i + 1) * 128, :])
        _emit_attention(nc, cx, 1)
        for tt in range(16):
            _emit_ph4_tile(nc, cx, tt)


_PROGRAM = None


def build_program():
    global _PROGRAM
    if _PROGRAM is not None:
        return _PROGRAM
    nc = bacc.Bacc("TRN2", target_bir_lowering=False, debug=False, num_devices=8)
    names = [("wq", [HID, D], F16), ("wk", [HID, D], F16),
             ("wv", [HID, DV], F16), ("wg", [HID, DV], F16),
             ("wqm", [D, D * R], F16),
             ("hst", [HID, T], F16),
             ("rw", [128, 16 * R], F32), ("nsel", [128, 16 * NE], F16),
             ("mbk", [128, NE * NB * TB], F16), ("wo", [DV, HID], F16)]
    io = [nc.dram_tensor(n, s, dt, kind="ExternalInput").ap() for n, s, dt in names]
    io.append(nc.dram_tensor("out", [T, HID], F16, kind="ExternalOutput").ap())
    with tile.TileContext(nc) as tc:
        from contextlib import ExitStack as ES
        with ES() as ctx:
            _body(ctx, nc, tc, io)
    nc.compile()
    _PROGRAM = nc
    return nc


def _host_routing(hs64, Wq64, Wgate64):
    """Per-head routing on host, matching the reference in f64.

    Returns rw [T, R] f32, msel [T, NE] f16 (1 active / 0 masked)."""
    S, K = 2, 2
    T_, _ = hs64.shape
    rw = np.zeros((T_, R), np.float64)
    rw[:, :S] = 0.25
    logits = hs64 @ (Wq64 @ Wgate64)          # [T, R-S]
    sc = np.exp(logits - logits.max(axis=-1, keepdims=True))
    sc /= sc.sum(axis=-1, keepdims=True)
    idx = np.argsort(-sc, axis=-1, kind="stable")[:, :K]   # top-2, ties -> low idx
    w = np.take_along_axis(sc, idx, axis=-1)
    w /= w.sum(axis=-1, keepdims=True)
    np.put_along_axis(rw[:, S:], idx, w * 0.5, axis=-1)
    msel = np.zeros((T_, NE), np.float64)
    np.put_along_axis(msel, idx, 1.0, axis=-1)
    return rw.astype(np.float32), msel.astype(np.float16)


def _assign_chunks(msel_b):
    """Assign each of the 1024 tokens to a 128-chunk such that both of its
    selected experts' windows contain the chunk.  Exact integer max-flow on
    the 6-pair x 4-super-chunk transportation graph (always feasible for
    the ~Binomial(1024, 1/6) group sizes this routing produces).

    Returns perm [1024]: position -> original token index."""
    sel = msel_b.astype(bool)
    pair_of = np.zeros(TB, np.int64)
    for p, (i_, j_) in enumerate(PAIRS):
        pair_of[sel[:, i_ - 2] & sel[:, j_ - 2]] = p
    n = np.bincount(pair_of, minlength=6)
    # Edmonds-Karp max-flow: node 0 = source, 1..6 = pairs, 7..10 = super-
    # chunks, 11 = sink.  source->pair cap n[p]; pair->sc cap inf; sc->sink 256.
    NV = 12
    cap = np.zeros((NV, NV), np.int64)
    for p in range(6):
        cap[0, 1 + p] = n[p]
        for s in REGION[p]:
            cap[1 + p, 7 + s] = TB
    for s in range(4):
        cap[7 + s, 11] = TB // 4
    total = 0
    while True:
        # BFS for augmenting path
        par = [-1] * NV
        par[0] = 0
        queue = [0]
        while queue:
            u = queue.pop(0)
            for v in range(NV):
                if par[v] < 0 and cap[u, v] > 0:
                    par[v] = u
                    queue.append(v)
        if par[11] < 0:
            break
        # min residual along path
        v, aug = 11, 1 << 40
        while v != 0:
            u = par[v]
            aug = min(aug, cap[u, v])
            v = u
        v = 11
        while v != 0:
            u = par[v]
            cap[u, v] -= aug
            cap[v, u] += aug
            v = u
        total += aug
    assert total == TB, f"chunk assignment infeasible (flow {total})"
    x = np.zeros((6, 4), np.int64)
    for p in range(6):
        for s in REGION[p]:
            x[p, s] = cap[7 + s, 1 + p]   # reverse edge = flow
    # build permutation: tokens of pair p fill super-chunks per x[p, :]
    sc_members = [[] for _ in range(4)]
    for p in range(6):
        toks = np.nonzero(pair_of == p)[0]
        o = 0
        for s in range(4):
            sc_members[s].extend(toks[o:o + x[p, s]])
            o += x[p, s]
    perm = np.concatenate([np.asarray(sc_members[s], np.int64) for s in range(4)])
    assert perm.shape[0] == TB
    return perm


def make_in_maps(hidden_states, Wq, Wk, Wv, Wq_exp, Wk_exp, Wgate, Wg, Wo):
    hs2 = np.asarray(hidden_states, np.float32).reshape(T, HID)
    hsT = np.ascontiguousarray(hs2.T.astype(np.float16))
    hs64 = hs2.astype(np.float64)
    Wq64 = np.asarray(Wq, np.float64)
    Wg64 = np.asarray(Wgate, np.float64)
    Wqe64 = np.asarray(Wq_exp, np.float64)
    Wke64 = np.asarray(Wk_exp, np.float64)
    in_maps = []
    perms = []
    # chunk index -> window membership per routed expert, for nsel'
    in_win = np.zeros((8, NE), bool)
    for r in WIN:
        for gc in WIN[r]:
            in_win[gc, r - 2] = True
    for c in range(8):
        rw, msel = _host_routing(hs64, Wq64[:, c * D:(c + 1) * D], Wg64)
        perm = np.empty(T, np.int64)
        for b in range(NB):
            pb = _assign_chunks(msel[b * TB:(b + 1) * TB])
            perm[b * TB:(b + 1) * TB] = b * TB + pb
        perms.append(perm)
        rw_p = rw[perm]
        msel_p = msel[perm].astype(np.float32)
        # nsel': -1 only for masked keys inside the expert's window
        chunk_of = (np.arange(T) % TB) // 128   # chunk within batch
        nsel_p = (msel_p - 1) * in_win[chunk_of, :]
        rw_sb = np.ascontiguousarray(
            rw_p.reshape(16, 128, R).transpose(1, 0, 2).reshape(128, 16 * R))
        nsel_sb = np.ascontiguousarray(
            nsel_p.reshape(16, 128, NE).transpose(1, 0, 2).reshape(128, 16 * NE)
        ).astype(np.float16)
        mbk = np.empty((128, NE * NB * TB), np.float16)
        for i in range(NE):
            for b in range(NB):
                mbk[:, (i * NB + b) * TB:(i * NB + b + 1) * TB] = \
                    msel_p[b * TB:(b + 1) * TB, i][None, :]
        wqm = np.empty((D, D * R), np.float16)
        for r in range(R):
            m = Wqe64[c][:, r * D:(r + 1) * D] @ Wke64[c][:, r * D:(r + 1) * D].T
            wqm[:, r * D:(r + 1) * D] = m.astype(np.float16)
        in_maps.append({
            "wq": np.asarray(Wq, np.float16)[:, c * D:(c + 1) * D].copy(),
            "wk": np.asarray(Wk, np.float16)[:, c * D:(c + 1) * D].copy(),
            "wv": np.asarray(Wv, np.float16)[:, c * DV:(c + 1) * DV].copy(),
            "wg": np.asarray(Wg, np.float16)[:, c * DV:(c + 1) * DV].copy(),
            "wqm": wqm,
            "hst": np.ascontiguousarray(hsT[:, perm]),
            "rw": rw_sb, "nsel": nsel_sb, "mbk": mbk,
            "wo": np.asarray(Wo, np.float16)[c * DV:(c + 1) * DV, :].copy(),
        })
    return in_maps, perms


def kernel(hidden_states, Wq, Wk, Wv, Wq_exp, Wk_exp, Wgate, Wg, Wo):
    nc = build_program()
    in_maps, perms = make_in_maps(hidden_states, Wq, Wk, Wv, Wq_exp, Wk_exp,
                                  Wgate, Wg, Wo)
    res = run_bass_kernel_spmd(nc, in_maps, list(range(8))).results
    out = np.zeros((T, HID), np.float32)
    for c in range(8):
        out[perms[c]] += res[c]["out"].astype(np.float32)
    return out.reshape(2, 1024, HID).astype(np.float32)
